# revision 17
# baseline (speedup 1.0000x reference)
"""AttentionPooling (segment softmax-pool) Trainium2 kernel, 8-way data parallel.

Math: s = x@W (+b, which cancels under softmax); g = softmax(s) over all N;
then a per-segment softmax of g pools x:
    pooled[seg] = sum_i x_i * exp(g_i) / sum_j exp(g_j).
Since g is a softmax output, g_i <= g_max ~ 1e-4 here, so
exp(g) = 1 + g + O(g^2) with relative error ~1e-8:
    pooled[seg] ~= (S + c*A) / (n + c*a),   c = 1/Z,  Z = sum_i exp(s_i)
where S/n are plain per-segment sums/counts and A/a are exp(s)-weighted.
Both accumulator pairs are linear in x, so the whole thing needs ONE pass
over x, and c is applied on the host after a trivial 8-way scalar gather.

Device layout (per core = 512 consecutive segments, C=16 chunks of
SEGC=32 segs; no collectives needed):
- Scores on the TensorEngine: host supplies xT in fp8 (grouped so MM p
  covers nodes {128*t + p}); W sits in shifted columns of 32-wide bf16
  weight tiles so tile_position col-strips land s directly in a [128, T]
  PSUM bank matching the node-tile layout.  One ScalarE Exp produces
  es=[128,T] f32 plus the per-partition Z partial (accum_out).
- Pooling: per 8-tile slab, stacked one-hots [oh | oh*es] (SEGC plain +
  SEGC exp-scaled cols per tile) are built in just 2 DVE tensor_tensor
  ops using stride-0 broadcast APs of idx/es columns; each 128-node tile
  then needs ONE bf16 matmul with rhs=[x|1], accumulating S,n (psum rows
  0:SEGC) and A,a (rows SEGC:2*SEGC) for its chunk.
- Host: Z = sum of zout, c = 1/Z, out = (S+c*A)/max(n+c*a, 0.5).
Engine budget per 128-node tile: DMA 184 ns (bf16 x, the roofline),
PE ~110 ns pool MM + ~107 ns score share, DVE ~140 ns, ACT ~0.
"""

import math

import numpy as np
import ml_dtypes

import concourse.bass as bass
import concourse.tile as tile
from concourse import bacc, mybir, bass_utils
from contextlib import ExitStack

P = 128
D = 256
XC = 258  # x (256) | ones | pad
NCORES = 8
NSEG = 4096
SEGC = 32  # segments per chunk (stacked one-hot: SEGC plain + SEGC scaled)
OHW = 2 * SEGC  # one-hot width per tile
SENT = 500.0
PAD_SCALE = -30.0  # xs pad columns = PAD_SCALE*sign(W) => s ~ -300 => exp=0

BF16 = ml_dtypes.bfloat16
FP8 = ml_dtypes.float8_e4m3fn

_prog_cache = {}

TRACE = False
LAST_EXEC_NS = None


def _plan(batch_idx):
    counts = np.bincount(batch_idx, minlength=NSEG)
    bounds = np.concatenate([[0], np.cumsum(counts)]).astype(np.int64)
    C = NSEG // NCORES // SEGC  # 8 chunks per core
    # Tc[j] = max over cores of tiles needed for chunk j
    Tc = []
    for j in range(C):
        mx = 0
        for k in range(NCORES):
            s0 = k * 512 + j * SEGC
            L = int(bounds[s0 + SEGC] - bounds[s0])
            mx = max(mx, math.ceil(L / P))
        Tc.append(mx)
    T = sum(Tc)
    assert T <= 512, f"T={T} exceeds PSUM bank"
    return bounds, C, Tc, T


def _build_core_inputs(x, batch_idx, W, bounds, core, C, Tc, T):
    Wf = W[:, 0].astype(np.float32)
    xperm = np.zeros((T * P, D), dtype=np.float32)
    ones = np.zeros((T * P,), dtype=np.float32)
    idxoff = np.full((T * P,), SENT, dtype=np.float32)
    tb = 0
    for j in range(C):
        s0 = core * 512 + j * SEGC
        m0, m1 = int(bounds[s0]), int(bounds[s0 + SEGC])
        L = m1 - m0
        r0 = tb * P
        xperm[r0:r0 + L] = x[m0:m1]
        ones[r0:r0 + L] = 1.0
        idxoff[r0:r0 + L] = (batch_idx[m0:m1] - s0).astype(np.float32)
        tb += Tc[j]
    # pooling operand: [128, T*258] bf16, partition-major
    xp3 = np.zeros((T * P, XC), dtype=np.float32)
    xp3[:, :D] = xperm
    xp3[:, D] = ones
    xp = np.ascontiguousarray(
        xp3.reshape(T, P, XC).transpose(1, 0, 2).reshape(P, T * XC)
    ).astype(BF16)
    # score operand: xT fp8, free order (p, h, t)
    xsrc = xperm
    pad = ones == 0.0
    if pad.any():
        xsrc = xperm.copy()
        xsrc[pad] = PAD_SCALE * np.sign(Wf)
    xs = np.ascontiguousarray(
        xsrc.reshape(T, P, 2, P).transpose(3, 1, 2, 0).reshape(P, P * 2 * T)
    ).astype(FP8)
    idxT = np.ascontiguousarray(idxoff.reshape(T, P).T)
    return {"xp": xp, "xs": xs, "idxT": idxT}


def _make_consts(W):
    Wf = W[:, 0].astype(np.float32)
    wvar = np.zeros((P, 2, 32, 32), dtype=np.float32)
    Wdh = Wf.reshape(2, P).T  # [d, h]
    k = np.arange(32)
    wvar[:, :, k, k] = Wdh[:, :, None]
    wvar = wvar.reshape(P, 2048).astype(BF16)
    rowb8 = np.broadcast_to(
        np.tile((np.arange(OHW) % SEGC).astype(np.float32), 8), (P, 8 * OHW))
    rowb8 = np.ascontiguousarray(rowb8).astype(BF16)
    return wvar, rowb8


def _build_program(C, Tc):
    T = sum(Tc)
    f32 = mybir.dt.float32
    bf16 = mybir.dt.bfloat16
    fp8 = mybir.dt.float8e4
    Alu = mybir.AluOpType
    Act = mybir.ActivationFunctionType

    nc = bacc.Bacc("TRN2", target_bir_lowering=False, debug=False)
    xp = nc.dram_tensor("xp", [P, T * XC], bf16, kind="ExternalInput").ap()
    xs = nc.dram_tensor("xs", [P, P * 2 * T], fp8, kind="ExternalInput").ap()
    idxT = nc.dram_tensor("idxT", [P, T], f32, kind="ExternalInput").ap()
    wvar = nc.dram_tensor("wvar", [P, 2048], bf16, kind="ExternalInput").ap()
    rowb8 = nc.dram_tensor("rowb8", [P, 8 * OHW], bf16, kind="ExternalInput").ap()
    pout = nc.dram_tensor("pout", [C * OHW, XC - 1], f32, kind="ExternalOutput").ap()
    zout = nc.dram_tensor("zout", [P, 1], f32, kind="ExternalOutput").ap()

    with tile.TileContext(nc) as tc, ExitStack() as ctx:
        const = ctx.enter_context(tc.tile_pool(name="const", bufs=1))
        wv_sb = const.tile([P, 2048], bf16, tag="wv")
        rb_sb = const.tile([P, 8 * OHW], bf16, tag="rb")
        ix_sb = const.tile([P, T], f32, tag="ix")
        es_sb = const.tile([P, T], f32, tag="es")
        zc_sb = const.tile([P, 1], f32, tag="zc")

        nc.scalar.dma_start(wv_sb[:], wvar[:, :])
        nc.scalar.dma_start(rb_sb[:], rowb8[:, :])
        nc.scalar.dma_start(ix_sb[:], idxT[:, :])

        # ---- phase 1: scores via col-strip matmuls ----
        xspool = ctx.enter_context(tc.tile_pool(name="xs", bufs=8))
        spspool = ctx.enter_context(
            tc.tile_pool(name="sps", bufs=1, space="PSUM"))
        sps = spspool.tile([P, T], f32, tag="sps")
        for blk in range(32):
            xsb = xspool.tile([P, 8 * T], fp8, tag="xsb")
            nc.scalar.dma_start(xsb[:], xs[:, blk * 8 * T:(blk + 1) * 8 * T])
            for g in range(4):
                p = blk * 4 + g
                j, k = p // 32, p % 32
                for h in (0, 1):
                    q = h * 32 + k
                    nc.tensor.matmul(
                        sps[32 * j:32 * j + 32, :],
                        lhsT=wv_sb[:, q * 32:(q + 1) * 32],
                        rhs=xsb[:, (g * 2 + h) * T:(g * 2 + h + 1) * T],
                        start=(k == 0 and h == 0),
                        stop=(k == 31 and h == 1),
                        tile_position=(0, 32 * j),
                    )
        nc.scalar.activation(es_sb[:], sps[:], Act.Exp, accum_out=zc_sb[:])
        nc.scalar.dma_start(zout[:, :], zc_sb[:])

        # ---- phase 2: stacked one-hot pooling matmuls ----
        xppool = ctx.enter_context(tc.tile_pool(name="xp", bufs=30))
        ohpool = ctx.enter_context(tc.tile_pool(name="oh", bufs=6))
        ppspool = ctx.enter_context(
            tc.tile_pool(name="pps", bufs=4, space="PSUM"))
        osbpool = ctx.enter_context(tc.tile_pool(name="osb", bufs=2))
        KSLAB = 8
        xpb = None
        ohb = None
        tbase = 0
        for c in range(C):
            pps = ppspool.tile([OHW, XC - 1], f32, tag="pps")
            for jt in range(Tc[c]):
                t = tbase + jt
                if t % KSLAB == 0:
                    w = min(KSLAB, T - t)
                    xpb = xppool.tile([P, KSLAB * XC], bf16, tag="xpb")
                    nc.sync.dma_start(
                        xpb[:, :w * XC], xp[:, t * XC:(t + w) * XC])
                    # stacked one-hots for the whole slab in 2 DVE ops
                    ohb = ohpool.tile([P, KSLAB * OHW], bf16, tag="oh")
                    oh3 = ohb[:, :w * OHW].rearrange("p (t c) -> p t c", t=w)
                    ixb = ix_sb[:, t:t + w].unsqueeze(-1).broadcast_to(
                        [P, w, OHW])
                    nc.vector.tensor_tensor(
                        out=oh3, in0=rb_sb[:, :w * OHW].rearrange(
                            "p (t c) -> p t c", t=w),
                        in1=ixb, op=Alu.is_equal)
                    esb = es_sb[:, t:t + w].unsqueeze(-1).broadcast_to(
                        [P, w, SEGC])
                    nc.vector.tensor_tensor(
                        out=oh3[:, :, SEGC:OHW], in0=oh3[:, :, SEGC:OHW],
                        in1=esb, op=Alu.mult)
                o = (t % KSLAB) * XC
                nc.tensor.matmul(
                    pps[:], lhsT=ohb[:, (t % KSLAB) * OHW:(t % KSLAB + 1) * OHW],
                    rhs=xpb[:, o:o + XC - 1],
                    start=(jt == 0), stop=(jt == Tc[c] - 1))
            osb = osbpool.tile([OHW, XC - 1], f32, tag="osb")
            nc.scalar.activation(osb[:], pps[:], Act.Identity)
            nc.scalar.dma_start(pout[c * OHW:(c + 1) * OHW, :], osb[:])
            tbase += Tc[c]

    nc.compile()
    return nc


def _get_program(C, Tc):
    key = (C, tuple(Tc))
    if key not in _prog_cache:
        _prog_cache[key] = _build_program(C, Tc)
    return _prog_cache[key]


def kernel(x, batch_idx, W, b, num_segments):
    x = np.asarray(x, dtype=np.float32)
    batch_idx = np.asarray(batch_idx)
    W = np.asarray(W, dtype=np.float32)
    assert int(num_segments) == NSEG and x.shape[1] == D

    bounds, C, Tc, T = _plan(batch_idx)
    nc = _get_program(C, Tc)

    wvar, rowb8 = _make_consts(W)
    in_maps = []
    for k in range(NCORES):
        m = _build_core_inputs(x, batch_idx, W, bounds, k, C, Tc, T)
        m["wvar"] = wvar
        m["rowb8"] = rowb8
        in_maps.append(m)

    global LAST_EXEC_NS
    res = bass_utils.run_bass_kernel_spmd(
        nc, in_maps, core_ids=list(range(NCORES)), trace=TRACE)
    if res.exec_time_ns is not None:
        LAST_EXEC_NS = res.exec_time_ns

    Z = np.float64(0.0)
    for k in range(NCORES):
        Z += res.results[k]["zout"].astype(np.float64).sum()
    c = np.float32(1.0 / Z)

    full = np.zeros((NSEG, D), dtype=np.float32)
    for k in range(NCORES):
        po = res.results[k]["pout"]
        for j in range(C):
            blk = po[j * OHW:(j + 1) * OHW]
            num = blk[0:SEGC, :D] + c * blk[SEGC:OHW, :D]
            den = np.maximum(blk[0:SEGC, D] + c * blk[SEGC:OHW, D], 0.5)
            s0 = k * 512 + j * SEGC
            full[s0:s0 + SEGC] = num / den[:, None]
    return full


# revision 18
# speedup vs baseline: 1.0607x; 1.0607x over previous
"""AttentionPooling (segment softmax-pool) Trainium2 kernel, 8-way data parallel.

Math: s = x@W (+b, which cancels under softmax); g = softmax(s) over all N;
then a per-segment softmax of g pools x:
    pooled[seg] = sum_i x_i * exp(g_i) / sum_j exp(g_j).
Since g is a softmax output, g_i <= g_max ~ 1e-4 here, so
exp(g) = 1 + g + O(g^2) with relative error ~1e-8:
    pooled[seg] ~= (S + c*A) / (n + c*a),   c = 1/Z,  Z = sum_i exp(s_i)
where S/n are plain per-segment sums/counts and A/a are exp(s)-weighted.
Both accumulator pairs are linear in x, so the whole thing needs ONE pass
over x, and c is applied on the host after a trivial 8-way scalar gather.

Device layout (per core = 512 consecutive segments, C=16 chunks of
SEGC=32 segs; no collectives needed):
- Scores on the TensorEngine: host supplies xT in fp8 (grouped so MM p
  covers nodes {128*t + p}); W sits in shifted columns of 32-wide bf16
  weight tiles so tile_position col-strips land s directly in a [128, T]
  PSUM bank matching the node-tile layout.  One ScalarE Exp produces
  es=[128,T] f32 plus the per-partition Z partial (accum_out).
- Pooling: per 8-tile slab, stacked one-hots [oh | oh*es] (SEGC plain +
  SEGC exp-scaled cols per tile) are built in just 2 DVE tensor_tensor
  ops using stride-0 broadcast APs of idx/es columns; each 128-node tile
  then needs ONE bf16 matmul with rhs=[x|1], accumulating S,n (psum rows
  0:SEGC) and A,a (rows SEGC:2*SEGC) for its chunk.
- Host: Z = sum of zout, c = 1/Z, out = (S+c*A)/max(n+c*a, 0.5).
Engine budget per 128-node tile: DMA 184 ns (bf16 x, the roofline),
PE ~110 ns pool MM + ~107 ns score share, DVE ~140 ns, ACT ~0.
"""

import math

import numpy as np
import ml_dtypes

import concourse.bass as bass
import concourse.tile as tile
from concourse import bacc, mybir, bass_utils
from contextlib import ExitStack

P = 128
D = 256
XC = 258  # x (256) | ones | pad
NCORES = 8
NSEG = 4096
SEGC = 32  # segments per chunk (stacked one-hot: SEGC plain + SEGC scaled)
OHW = 2 * SEGC  # one-hot width per tile
SENT = 500.0
PAD_SCALE = -30.0  # xs pad columns = PAD_SCALE*sign(W) => s ~ -300 => exp=0

BF16 = ml_dtypes.bfloat16
FP8 = ml_dtypes.float8_e4m3fn

_prog_cache = {}

TRACE = False
LAST_EXEC_NS = None


def _plan(batch_idx):
    counts = np.bincount(batch_idx, minlength=NSEG)
    bounds = np.concatenate([[0], np.cumsum(counts)]).astype(np.int64)
    C = NSEG // NCORES // SEGC  # 8 chunks per core
    # Tc[j] = max over cores of tiles needed for chunk j
    Tc = []
    for j in range(C):
        mx = 0
        for k in range(NCORES):
            s0 = k * 512 + j * SEGC
            L = int(bounds[s0 + SEGC] - bounds[s0])
            mx = max(mx, math.ceil(L / P))
        Tc.append(mx)
    T = sum(Tc)
    assert T <= 512, f"T={T} exceeds PSUM bank"
    return bounds, C, Tc, T


def _build_core_inputs(x, batch_idx, W, bounds, core, C, Tc, T):
    Wf = W[:, 0].astype(np.float32)
    xperm = np.zeros((T * P, D), dtype=np.float32)
    ones = np.zeros((T * P,), dtype=np.float32)
    idxoff = np.full((T * P,), SENT, dtype=np.float32)
    tb = 0
    for j in range(C):
        s0 = core * 512 + j * SEGC
        m0, m1 = int(bounds[s0]), int(bounds[s0 + SEGC])
        L = m1 - m0
        r0 = tb * P
        xperm[r0:r0 + L] = x[m0:m1]
        ones[r0:r0 + L] = 1.0
        idxoff[r0:r0 + L] = (batch_idx[m0:m1] - s0).astype(np.float32)
        tb += Tc[j]
    # pooling operand: [128, T*258] bf16, partition-major
    xp3 = np.zeros((T * P, XC), dtype=np.float32)
    xp3[:, :D] = xperm
    xp3[:, D] = ones
    xp = np.ascontiguousarray(
        xp3.reshape(T, P, XC).transpose(1, 0, 2).reshape(P, T * XC)
    ).astype(BF16)
    # score operand: xT fp8, free order (p, h, t)
    xsrc = xperm
    pad = ones == 0.0
    if pad.any():
        xsrc = xperm.copy()
        xsrc[pad] = PAD_SCALE * np.sign(Wf)
    xs = np.ascontiguousarray(
        xsrc.reshape(T, P, 2, P).transpose(3, 1, 2, 0).reshape(P, P * 2 * T)
    ).astype(FP8)
    idxT = np.ascontiguousarray(idxoff.reshape(T, P).T)
    return {"xp": xp, "xs": xs, "idxT": idxT}


def _make_consts(W):
    Wf = W[:, 0].astype(np.float32)
    wvar = np.zeros((P, 2, 32, 32), dtype=np.float32)
    Wdh = Wf.reshape(2, P).T  # [d, h]
    k = np.arange(32)
    wvar[:, :, k, k] = Wdh[:, :, None]
    wvar = wvar.reshape(P, 2048).astype(BF16)
    rowb8 = np.broadcast_to(
        np.tile((np.arange(OHW) % SEGC).astype(np.float32), 8), (P, 8 * OHW))
    rowb8 = np.ascontiguousarray(rowb8).astype(BF16)
    return wvar, rowb8


def _build_program(C, Tc):
    T = sum(Tc)
    f32 = mybir.dt.float32
    bf16 = mybir.dt.bfloat16
    fp8 = mybir.dt.float8e4
    Alu = mybir.AluOpType
    Act = mybir.ActivationFunctionType

    nc = bacc.Bacc("TRN2", target_bir_lowering=False, debug=False)
    xp = nc.dram_tensor("xp", [P, T * XC], bf16, kind="ExternalInput").ap()
    xs = nc.dram_tensor("xs", [P, P * 2 * T], fp8, kind="ExternalInput").ap()
    idxT = nc.dram_tensor("idxT", [P, T], f32, kind="ExternalInput").ap()
    wvar = nc.dram_tensor("wvar", [P, 2048], bf16, kind="ExternalInput").ap()
    rowb8 = nc.dram_tensor("rowb8", [P, 8 * OHW], bf16, kind="ExternalInput").ap()
    pout = nc.dram_tensor("pout", [C * OHW, XC - 1], f32, kind="ExternalOutput").ap()
    zout = nc.dram_tensor("zout", [P, 1], f32, kind="ExternalOutput").ap()

    with tile.TileContext(nc) as tc, ExitStack() as ctx:
        const = ctx.enter_context(tc.tile_pool(name="const", bufs=1))
        wv_sb = const.tile([P, 2048], bf16, tag="wv")
        rb_sb = const.tile([P, 8 * OHW], bf16, tag="rb")
        ix_sb = const.tile([P, T], f32, tag="ix")
        es_sb = const.tile([P, T], f32, tag="es")
        zc_sb = const.tile([P, 1], f32, tag="zc")

        nc.scalar.dma_start(wv_sb[:], wvar[:, :])
        nc.scalar.dma_start(rb_sb[:], rowb8[:, :])
        nc.scalar.dma_start(ix_sb[:], idxT[:, :])

        # ---- phase 1: scores via col-strip matmuls ----
        xspool = ctx.enter_context(tc.tile_pool(name="xs", bufs=6))
        spspool = ctx.enter_context(
            tc.tile_pool(name="sps", bufs=1, space="PSUM"))
        sps = spspool.tile([P, T], f32, tag="sps")
        for blk in range(32):
            xsb = xspool.tile([P, 8 * T], fp8, tag="xsb")
            nc.sync.dma_start(xsb[:], xs[:, blk * 8 * T:(blk + 1) * 8 * T])
            for g in range(4):
                p = blk * 4 + g
                j, k = p // 32, p % 32
                for h in (0, 1):
                    q = h * 32 + k
                    nc.tensor.matmul(
                        sps[32 * j:32 * j + 32, :],
                        lhsT=wv_sb[:, q * 32:(q + 1) * 32],
                        rhs=xsb[:, (g * 2 + h) * T:(g * 2 + h + 1) * T],
                        start=(k == 0 and h == 0),
                        stop=(k == 31 and h == 1),
                        tile_position=(0, 32 * j),
                    )
        nc.scalar.activation(es_sb[:], sps[:], Act.Exp, accum_out=zc_sb[:])
        nc.scalar.dma_start(zout[:, :], zc_sb[:])

        # ---- phase 2: stacked one-hot pooling matmuls ----
        xppool = ctx.enter_context(tc.tile_pool(name="xp", bufs=24))
        ohpool = ctx.enter_context(tc.tile_pool(name="oh", bufs=6))
        ppspool = ctx.enter_context(
            tc.tile_pool(name="pps", bufs=4, space="PSUM"))
        osbpool = ctx.enter_context(tc.tile_pool(name="osb", bufs=2))
        KSLAB = 8
        xpb = None
        ohb = None
        tbase = 0
        for c in range(C):
            pps = ppspool.tile([OHW, XC - 1], f32, tag="pps")
            for jt in range(Tc[c]):
                t = tbase + jt
                if t % KSLAB == 0:
                    w = min(KSLAB, T - t)
                    xpb = xppool.tile([P, KSLAB * XC], bf16, tag="xpb")
                    nc.sync.dma_start(
                        xpb[:, :w * XC], xp[:, t * XC:(t + w) * XC])
                    # stacked one-hots for the whole slab in 2 DVE ops
                    ohb = ohpool.tile([P, KSLAB * OHW], bf16, tag="oh")
                    oh3 = ohb[:, :w * OHW].rearrange("p (t c) -> p t c", t=w)
                    ixb = ix_sb[:, t:t + w].unsqueeze(-1).broadcast_to(
                        [P, w, OHW])
                    nc.vector.tensor_tensor(
                        out=oh3, in0=rb_sb[:, :w * OHW].rearrange(
                            "p (t c) -> p t c", t=w),
                        in1=ixb, op=Alu.is_equal)
                    esb = es_sb[:, t:t + w].unsqueeze(-1).broadcast_to(
                        [P, w, SEGC])
                    nc.vector.tensor_tensor(
                        out=oh3[:, :, SEGC:OHW], in0=oh3[:, :, SEGC:OHW],
                        in1=esb, op=Alu.mult)
                o = (t % KSLAB) * XC
                nc.tensor.matmul(
                    pps[:], lhsT=ohb[:, (t % KSLAB) * OHW:(t % KSLAB + 1) * OHW],
                    rhs=xpb[:, o:o + XC - 1],
                    start=(jt == 0), stop=(jt == Tc[c] - 1))
            osb = osbpool.tile([OHW, XC - 1], f32, tag="osb")
            nc.scalar.activation(osb[:], pps[:], Act.Identity)
            nc.scalar.dma_start(pout[c * OHW:(c + 1) * OHW, :], osb[:])
            tbase += Tc[c]

    nc.compile()
    return nc


def _get_program(C, Tc):
    key = (C, tuple(Tc))
    if key not in _prog_cache:
        _prog_cache[key] = _build_program(C, Tc)
    return _prog_cache[key]


def kernel(x, batch_idx, W, b, num_segments):
    x = np.asarray(x, dtype=np.float32)
    batch_idx = np.asarray(batch_idx)
    W = np.asarray(W, dtype=np.float32)
    assert int(num_segments) == NSEG and x.shape[1] == D

    bounds, C, Tc, T = _plan(batch_idx)
    nc = _get_program(C, Tc)

    wvar, rowb8 = _make_consts(W)
    in_maps = []
    for k in range(NCORES):
        m = _build_core_inputs(x, batch_idx, W, bounds, k, C, Tc, T)
        m["wvar"] = wvar
        m["rowb8"] = rowb8
        in_maps.append(m)

    global LAST_EXEC_NS
    res = bass_utils.run_bass_kernel_spmd(
        nc, in_maps, core_ids=list(range(NCORES)), trace=TRACE)
    if res.exec_time_ns is not None:
        LAST_EXEC_NS = res.exec_time_ns

    Z = np.float64(0.0)
    for k in range(NCORES):
        Z += res.results[k]["zout"].astype(np.float64).sum()
    c = np.float32(1.0 / Z)

    full = np.zeros((NSEG, D), dtype=np.float32)
    for k in range(NCORES):
        po = res.results[k]["pout"]
        for j in range(C):
            blk = po[j * OHW:(j + 1) * OHW]
            num = blk[0:SEGC, :D] + c * blk[SEGC:OHW, :D]
            den = np.maximum(blk[0:SEGC, D] + c * blk[SEGC:OHW, D], 0.5)
            s0 = k * 512 + j * SEGC
            full[s0:s0 + SEGC] = num / den[:, None]
    return full
